# revision 1
# baseline (speedup 1.0000x reference)
"""Trainium2 Bass kernel for nn_CustomAttention (B=4, N=2048, DIM=1024, 16 heads x 64).

Sharding: 8 cores = 4 batches x 2 head-groups (8 heads each).
Per core: QKV projection for its 8 heads, attention, partial out-projection
(its 512 rows of w_out). Host sums the two partial outputs per batch + bias.

Layout strategy (all matmuls fp32r, 1 cyc/row, contraction on partitions):
 - xT [DIM, N] resident in SBUF during projections (host pre-transposes x[b]).
 - Q^T/K^T per head-pair [128, N] = W_slice.T @ xT  (pair packs 2 heads' d=64).
 - S^T[k_tile, q] = K^T-slice.T @ Q^T  (contraction d=64; the pair's two heads
   run concurrently in PE row-groups 0-63 / 64-127).
 - E = exp(S * scale) on ACT directly from PSUM, fp32r to SBUF.
   (no max-subtraction: scores are ~N(0,1) after scaling, exp cannot overflow)
 - O' [65, q] += [V|1].T @ E  accumulated over key tiles in PSUM; row 64 is the
   softmax denominator (ones-column trick).
 - normalize: reciprocal on DVE, partition-broadcast on GpSimd, multiply on DVE.
 - out-projection: y[tok, dim] += A^T-slice.T @ w_out_slice.
"""

import sys

sys.path.insert(0, '/opt/trn_rl_repo')

import numpy as np

import concourse.bass as bass
import concourse.tile as tile
from concourse import bacc, mybir
from concourse.bass_utils import run_bass_kernel_spmd

B, N_TOK, DIM = 4, 2048, 1024
HEADS_TOTAL, D_HEAD = 16, 64
G_HEADS = 8              # heads per core
PAIRS = G_HEADS // 2     # head pairs per core
INNER_G = G_HEADS * D_HEAD   # 512, inner slice per core
SCALE = D_HEAD ** -0.5
F32 = mybir.dt.float32
F32R = mybir.dt.float32r

_NC_CACHE = {}


def build_kernel(n_tok=N_TOK, repeat=1, parts="all"):
    nc = bacc.Bacc("TRN2")
    xt = nc.declare_dram_parameter("xt", [DIM, n_tok], F32, isOutput=False)
    wq = nc.declare_dram_parameter("wq", [DIM, INNER_G], F32, isOutput=False)
    wk = nc.declare_dram_parameter("wk", [DIM, INNER_G], F32, isOutput=False)
    wv = nc.declare_dram_parameter("wv", [DIM, INNER_G], F32, isOutput=False)
    wo = nc.declare_dram_parameter("wo", [INNER_G, DIM], F32, isOutput=False)
    y = nc.declare_dram_parameter("y", [n_tok, DIM], F32, isOutput=True)

    KD = DIM // 128          # 8 contraction tiles for projections
    NQC = max(1, n_tok // 512)       # 512-wide chunks of tokens
    QCW = n_tok // NQC               # token chunk width (<=512)
    NTT = n_tok // 128       # 128-wide token tiles
    HALF = n_tok // 2        # q-half processed per PSUM pass
    NHC = max(1, HALF // 512)        # 512-wide chunks within a half
    HCW = HALF // NHC                # half chunk width (<=512)
    KT = n_tok // 128        # key tiles in attention

    import contextlib

    with tile.TileContext(nc) as tc:
      with (tc.For_i(0, repeat, 1) if repeat > 1 else contextlib.nullcontext()):
        with tc.tile_pool(name="persist", bufs=1) as persist:
            qT = [persist.tile([128, n_tok], F32R, name=f"qT{p}") for p in range(PAIRS)]
            kT = [persist.tile([128, n_tok], F32R, name=f"kT{p}") for p in range(PAIRS)]
            vb = [persist.tile([128, G_HEADS, D_HEAD + 1], F32R, name=f"vb{t}")
                  for t in range(NTT)]

            # ---------------- Phase 1: projections (xT resident) ----------------
            with tc.tile_pool(name="ph1", bufs=1) as ph1, \
                 tc.tile_pool(name="wqk", bufs=2) as wqk:
                xt_sb = [ph1.tile([128, n_tok], F32R, name=f"xt{i}") for i in range(KD)]
                for i in range(KD):
                    nc.sync.dma_start(out=xt_sb[i],
                                      in_=xt[i * 128:(i + 1) * 128, :].bitcast(F32R))
                ones8_f32 = ph1.tile([128, G_HEADS], F32, name="ones8_f32")
                nc.vector.memset(ones8_f32, 1.0)
                ones8 = ph1.tile([128, G_HEADS], F32R, name="ones8")
                nc.vector.tensor_copy(out=ones8, in_=ones8_f32)

                # V projection
                with tc.tile_pool(name="wvpool", bufs=1) as wvpool, \
                     tc.tile_pool(name="ps1", bufs=2, space="PSUM") as ps1:
                    wv_sb = [wvpool.tile([128, INNER_G], F32R, name=f"wv{i}")
                             for i in range(KD)]
                    for i in range(KD):
                        nc.sync.dma_start(out=wv_sb[i],
                                          in_=wv[i * 128:(i + 1) * 128, :].bitcast(F32R))
                    for t in range(NTT if parts != "dma" else 0):
                        vps = ps1.tile([128, INNER_G], F32, tag="vps")
                        for i in range(KD):
                            nc.tensor.matmul(out=vps,
                                             lhsT=xt_sb[i][:, t * 128:(t + 1) * 128],
                                             rhs=wv_sb[i],
                                             start=(i == 0), stop=(i == KD - 1))
                        nc.vector.tensor_copy(out=vb[t][:, :, D_HEAD], in_=ones8)
                        nc.vector.tensor_copy(
                            out=vb[t][:, :, 0:D_HEAD],
                            in_=vps.rearrange("p (h d) -> p h d", h=G_HEADS))

                # QK^T projections, per pair (2 heads = 128 W columns)
                with tc.tile_pool(name="ps2", bufs=2, space="PSUM") as ps2:
                    for p in range(PAIRS):
                        for (wt, dst) in ((wq, qT[p]), (wk, kT[p])):
                            wtiles = []
                            for i in range(KD):
                                wti = wqk.tile([128, 128], F32R, tag=f"w{i}",
                                               name=f"w_{p}_{i}")
                                nc.sync.dma_start(
                                    out=wti,
                                    in_=wt[i * 128:(i + 1) * 128,
                                           p * 128:(p + 1) * 128].bitcast(F32R))
                                wtiles.append(wti)
                            for qc in range(NQC if parts != "dma" else 0):
                                pqk = ps2.tile([128, QCW], F32, tag="pqk")
                                for i in range(KD):
                                    nc.tensor.matmul(
                                        out=pqk,
                                        lhsT=wtiles[i],
                                        rhs=xt_sb[i][:, qc * QCW:(qc + 1) * QCW],
                                        start=(i == 0), stop=(i == KD - 1))
                                nc.vector.tensor_copy(
                                    out=dst[:, qc * QCW:(qc + 1) * QCW], in_=pqk)

            # ---------------- Phase 2+3: attention, out-projection ----------------
            with tc.tile_pool(name="late", bufs=1) as late:
                aT = [late.tile([128, n_tok], F32R, name=f"aT{p}") for p in range(PAIRS)]
                if parts == "noatt":
                    for p in range(PAIRS):
                        nc.vector.memset(aT[p].bitcast(F32), 0.0)

                with tc.tile_pool(name="att_ps", bufs=1, space="PSUM") as att_ps, \
                     tc.tile_pool(name="att_sb", bufs=2) as att_sb, \
                     tc.tile_pool(name="norm_sb", bufs=2) as norm_sb:
                    for p in range(PAIRS if parts in ("all", "noout") else 0):
                        for qh in range(2):
                            q0 = qh * HALF
                            s_ps = [att_ps.tile([128, HALF], F32, tag=f"s{hl}",
                                                name=f"s_{p}_{qh}_{hl}")
                                    for hl in range(2)]
                            o_ps = [att_ps.tile([D_HEAD + 1, HALF], F32, tag=f"o{hl}",
                                                name=f"o_{p}_{qh}_{hl}")
                                    for hl in range(2)]
                            # software pipeline: S/exp for kt, O for kt-1, so
                            # the in-order PE queue never blocks next S behind
                            # an O that waits on the current exp.
                            e_prev = None
                            for kt_i in range(KT):
                                e_sb = []
                                for hl in range(2):
                                    po = hl * 64
                                    for c in range(NHC):
                                        nc.tensor.matmul(
                                            out=s_ps[hl][:, c * HCW:(c + 1) * HCW],
                                            lhsT=kT[p][po:po + 64,
                                                       kt_i * 128:(kt_i + 1) * 128],
                                            rhs=qT[p][po:po + 64,
                                                      q0 + c * HCW:q0 + (c + 1) * HCW],
                                            start=True, stop=True)
                                    et = att_sb.tile([128, HALF], F32R, tag=f"e{hl}",
                                                     name=f"e_{p}_{qh}_{kt_i}_{hl}")
                                    nc.scalar.activation(
                                        out=et, in_=s_ps[hl],
                                        func=mybir.ActivationFunctionType.Exp,
                                        scale=SCALE)
                                    e_sb.append(et)
                                if e_prev is not None:
                                    for hl in range(2):
                                        for c in range(NHC):
                                            nc.tensor.matmul(
                                                out=o_ps[hl][:, c * HCW:(c + 1) * HCW],
                                                lhsT=vb[kt_i - 1][:, 2 * p + hl, :],
                                                rhs=e_prev[hl][:, c * HCW:(c + 1) * HCW],
                                                start=(kt_i == 1), stop=False)
                                e_prev = e_sb
                            for hl in range(2):
                                for c in range(NHC):
                                    nc.tensor.matmul(
                                        out=o_ps[hl][:, c * HCW:(c + 1) * HCW],
                                        lhsT=vb[KT - 1][:, 2 * p + hl, :],
                                        rhs=e_prev[hl][:, c * HCW:(c + 1) * HCW],
                                        start=False, stop=True)
                            # normalize by the ones-column row sums
                            for hl in range(2):
                                rt = norm_sb.tile([D_HEAD + 1, HALF], F32, tag="rt",
                                                  name=f"rt_{p}_{qh}_{hl}")
                                nc.vector.reciprocal(out=rt[64:65, :],
                                                     in_=o_ps[hl][64:65, :])
                                r0 = norm_sb.tile([1, HALF], F32, tag="r0",
                                                  name=f"r0_{p}_{qh}_{hl}")
                                nc.sync.dma_start(out=r0, in_=rt[64:65, :])
                                rb = norm_sb.tile([64, HALF], F32, tag="rb",
                                                  name=f"rb_{p}_{qh}_{hl}")
                                nc.gpsimd.partition_broadcast(rb, r0)
                                if hl == 0:
                                    nc.vector.tensor_mul(
                                        aT[p][0:64, q0:q0 + HALF],
                                        o_ps[hl][0:64, :], rb)
                                else:
                                    tmpb = norm_sb.tile([64, HALF], F32R, tag="tmpb",
                                                        name=f"tmpb_{p}_{qh}")
                                    nc.vector.tensor_mul(tmpb, o_ps[hl][0:64, :], rb)
                                    nc.sync.dma_start(
                                        out=aT[p][64:128, q0:q0 + HALF], in_=tmpb)

                # out projection
                with tc.tile_pool(name="wopool", bufs=1) as wopool, \
                     tc.tile_pool(name="ps3", bufs=2, space="PSUM") as ps3, \
                     tc.tile_pool(name="ysb", bufs=3) as ysb:
                    wo_sb = [wopool.tile([128, DIM], F32R, name=f"wo{j}")
                             for j in range(PAIRS)]
                    for j in range(PAIRS):
                        nc.sync.dma_start(out=wo_sb[j],
                                          in_=wo[j * 128:(j + 1) * 128, :].bitcast(F32R))
                    for t in range(NTT):
                        for dc in range(2):
                            yt = ysb.tile([128, 512], F32, tag="yt")
                            if parts in ("dma", "noout"):
                                nc.vector.memset(yt, 0.0)
                            else:
                                yps = ps3.tile([128, 512], F32, tag="yps")
                                for j in range(PAIRS):
                                    nc.tensor.matmul(
                                        out=yps,
                                        lhsT=aT[j][:, t * 128:(t + 1) * 128],
                                        rhs=wo_sb[j][:, dc * 512:(dc + 1) * 512],
                                        start=(j == 0), stop=(j == PAIRS - 1))
                                nc.vector.tensor_copy(out=yt, in_=yps)
                            nc.sync.dma_start(
                                out=y[t * 128:(t + 1) * 128,
                                      dc * 512:(dc + 1) * 512],
                                in_=yt)

    nc.compile()
    return nc


def kernel(x, w_qkv, w_out, b_out):
    x = np.asarray(x, dtype=np.float32)
    w_qkv = np.asarray(w_qkv, dtype=np.float32)
    w_out = np.asarray(w_out, dtype=np.float32)
    b_out = np.asarray(b_out, dtype=np.float32)

    if N_TOK not in _NC_CACHE:
        _NC_CACHE[N_TOK] = build_kernel(N_TOK)
    nc = _NC_CACHE[N_TOK]

    core_ids = list(range(8))
    in_maps = _make_in_maps(x, w_qkv, w_out)
    res = run_bass_kernel_spmd(nc, in_maps, core_ids)
    out = np.empty((B, N_TOK, DIM), dtype=np.float32)
    for b in range(B):
        out[b] = res.results[2 * b]["y"] + res.results[2 * b + 1]["y"] + b_out
    return out


def _make_in_maps(x, w_qkv, w_out):
    in_maps = []
    for c in range(8):
        b, g = c // 2, c % 2
        sl = slice(g * INNER_G, (g + 1) * INNER_G)
        in_maps.append({
            "xt": np.ascontiguousarray(x[b].T),
            "wq": np.ascontiguousarray(w_qkv[:, 0 * DIM + sl.start:0 * DIM + sl.stop]),
            "wk": np.ascontiguousarray(w_qkv[:, 1 * DIM + sl.start:1 * DIM + sl.stop]),
            "wv": np.ascontiguousarray(w_qkv[:, 2 * DIM + sl.start:2 * DIM + sl.stop]),
            "wo": np.ascontiguousarray(w_out[sl]),
        })
    return in_maps



# revision 8
# speedup vs baseline: 49.5943x; 49.5943x over previous
"""Trainium2 Bass kernel for nn_CustomAttention (B=4, N=2048, DIM=1024, 16 heads x 64).

Sharding: 8 cores = 4 batches x 2 head-groups (8 heads each).
Per core: QKV projection for its 8 heads, attention, partial out-projection
(its 512 rows of w_out). Host sums the two partial outputs per batch + bias.

v3 schedule: single open scope, all matmul inputs bf16 (host-converted;
~7e-3 rel err vs 2e-2 tolerance), PSUM accumulate f32. The softmax exp on
ACT (~266us/core) and the matmuls on PE (~333us/core) are both near their
hw floors; the schedule keeps PE 100% busy and hides everything else:

  prologue: DMA in; project kT[0] (full) + qT[0] (first q-half) inline
  attention units (head x q-half), S(kt)->exp(kt)->O(kt-1) pipeline with
  per-unit work-queue pops injected into the PE stream:
    unit 0: V-projection tiles (O(kt) gated on vb[kt], pops stay ahead)
    unit 1: rest of qT[0] + QK-proj pair 1
    units 2-7: QK-proj pairs 2-3
    qh=1 units: out-projection of qh=0 tokens
  epilogue: out-projection of qh=1 tokens

S^T[k,q] per head contracts d=64 (kT/qT pair tiles hold 2 heads in
partition halves); O'[65,q] accumulates [V|1].T @ E over key tiles in PSUM
(row 64 = softmax denominator); normalize = copy to SBUF (frees the single
o_ps buffer) -> row DMA -> reciprocal -> partition broadcast -> multiply;
out-proj y[tok,dim] += aT.T @ wo per 128-token tile.
"""

import sys

sys.path.insert(0, '/opt/trn_rl_repo')

import numpy as np
import ml_dtypes

import concourse.bass as bass
import concourse.tile as tile
from concourse import bacc, mybir
from concourse.bass_utils import run_bass_kernel_spmd

B, N_TOK, DIM = 4, 2048, 1024
HEADS_TOTAL, D_HEAD = 16, 64
G_HEADS = 8              # heads per core
PAIRS = G_HEADS // 2     # head pairs per core
INNER_G = G_HEADS * D_HEAD   # 512, inner slice per core
SCALE = D_HEAD ** -0.5
F32 = mybir.dt.float32
BF16 = mybir.dt.bfloat16
BF16_NP = ml_dtypes.bfloat16

_NC_CACHE = {}

# queue pops per kt step, by attention unit index (8 units per q-half)
POPS_QH0 = (10, 3, 2, 2, 1, 2, 2, 2)
POPS_QH1 = (1, 1, 1, 1, 1, 1, 1, 1)


def build_kernel(n_tok=N_TOK, repeat=1, pops_qh0=POPS_QH0, pops_qh1=POPS_QH1,
                 ebufs=4):
    nc = bacc.Bacc("TRN2")
    xt = nc.declare_dram_parameter("xt", [DIM, n_tok], BF16, isOutput=False)
    wq = nc.declare_dram_parameter("wq", [DIM, INNER_G], BF16, isOutput=False)
    wk = nc.declare_dram_parameter("wk", [DIM, INNER_G], BF16, isOutput=False)
    wv = nc.declare_dram_parameter("wv", [DIM, INNER_G], BF16, isOutput=False)
    wo = nc.declare_dram_parameter("wo", [INNER_G, DIM], BF16, isOutput=False)
    y = nc.declare_dram_parameter("y", [n_tok, DIM], F32, isOutput=True)

    KD = DIM // 128          # 8 contraction tiles for projections
    NTT = n_tok // 128       # 128-wide token tiles
    HALF = n_tok // 2        # q-half processed per attention unit
    KT = n_tok // 128        # key tiles in attention

    import contextlib

    with tile.TileContext(nc) as tc:
      with (tc.For_i(0, repeat, 1) if repeat > 1 else contextlib.nullcontext()):
        with tc.tile_pool(name="main", bufs=1) as mp, \
             tc.tile_pool(name="epool", bufs=ebufs) as ep, \
             tc.tile_pool(name="norm", bufs=1) as npool, \
             tc.tile_pool(name="ypool", bufs=2) as ypool, \
             tc.tile_pool(name="ps_s", bufs=2, space="PSUM") as ps_s, \
             tc.tile_pool(name="ps_o", bufs=1, space="PSUM") as ps_o, \
             tc.tile_pool(name="ps_pj", bufs=2, space="PSUM") as ps_pj:

            xt_sb = [mp.tile([128, n_tok], BF16, name=f"xt{i}") for i in range(KD)]
            wq_sb = [mp.tile([128, INNER_G], BF16, name=f"wq{i}") for i in range(KD)]
            wk_sb = [mp.tile([128, INNER_G], BF16, name=f"wk{i}") for i in range(KD)]
            wv_sb = [mp.tile([128, INNER_G], BF16, name=f"wv{i}") for i in range(KD)]
            wo_sb = [mp.tile([128, DIM], BF16, name=f"wo{j}") for j in range(PAIRS)]
            qT = [mp.tile([128, n_tok], BF16, name=f"qT{p}") for p in range(PAIRS)]
            kT = [mp.tile([128, n_tok], BF16, name=f"kT{p}") for p in range(PAIRS)]
            vb = [mp.tile([128, G_HEADS, D_HEAD + 1], BF16, name=f"vb{t}")
                  for t in range(NTT)]
            aT = [mp.tile([128, n_tok], BF16, name=f"aT{p}") for p in range(PAIRS)]
            ones8 = mp.tile([128, G_HEADS], BF16, name="ones8")

            # ---- input DMAs (sync queue; order gates first projections) ----
            for i in range(KD):
                nc.sync.dma_start(out=xt_sb[i], in_=xt[i * 128:(i + 1) * 128, :])
                nc.gpsimd.dma_start(out=wk_sb[i], in_=wk[i * 128:(i + 1) * 128, :])
                nc.gpsimd.dma_start(out=wq_sb[i], in_=wq[i * 128:(i + 1) * 128, :])
            for i in range(KD):
                nc.gpsimd.dma_start(out=wv_sb[i], in_=wv[i * 128:(i + 1) * 128, :])
            for j in range(PAIRS):
                nc.gpsimd.dma_start(out=wo_sb[j], in_=wo[j * 128:(j + 1) * 128, :])
            nc.vector.memset(ones8, 1.0)

            # ---- work items: each emits one 512-row matmul chunk ----
            def qk_chunk(w_sb, dst, p, qc, st, dname):
                # 8 accumulation items for one [128, 512] projection chunk
                for i in range(KD):
                    def item(i=i, w_sb=w_sb, dst=dst, qc=qc, st=st, p=p,
                             dname=dname):
                        if i == 0:
                            st['ps'] = ps_pj.tile(
                                [128, 512], F32, tag="pj",
                                name=f"pj_{dname}_{qc}")
                        nc.tensor.matmul(
                            out=st['ps'],
                            lhsT=w_sb[i][:, p * 128:(p + 1) * 128],
                            rhs=xt_sb[i][:, qc * 512:(qc + 1) * 512],
                            start=(i == 0), stop=(i == KD - 1))
                        if i == KD - 1:
                            nc.vector.tensor_copy(
                                out=dst[:, qc * 512:(qc + 1) * 512],
                                in_=st['ps'])
                    yield item

            def vproj_tile(t, st):
                for i in range(KD):
                    def item(i=i, t=t, st=st):
                        if i == 0:
                            st['ps'] = ps_pj.tile([128, 512], F32, tag="pj",
                                                  name=f"pj_v_{t}")
                        nc.tensor.matmul(
                            out=st['ps'],
                            lhsT=xt_sb[i][:, t * 128:(t + 1) * 128],
                            rhs=wv_sb[i],
                            start=(i == 0), stop=(i == KD - 1))
                        if i == KD - 1:
                            nc.vector.tensor_copy(out=vb[t][:, :, D_HEAD],
                                                  in_=ones8)
                            nc.vector.tensor_copy(
                                out=vb[t][:, :, 0:D_HEAD],
                                in_=st['ps'].rearrange("p (h d) -> p h d",
                                                       h=G_HEADS))
                    yield item

            def outproj_tile(t, st):
                for dc in range(2):
                    for j in (3, 2, 1, 0):
                        def item(t=t, dc=dc, j=j, st=st):
                            if dc == 0 and j == 3:
                                st['y'] = ypool.tile([128, DIM], F32, tag="y",
                                                     name=f"y_{t}")
                            if j == 3:
                                st['ps'] = ps_pj.tile(
                                    [128, 512], F32, tag="pj",
                                    name=f"pj_out_{t}_{dc}")
                            nc.tensor.matmul(
                                out=st['ps'],
                                lhsT=aT[j][:, t * 128:(t + 1) * 128],
                                rhs=wo_sb[j][:, dc * 512:(dc + 1) * 512],
                                start=(j == 3), stop=(j == 0))
                            if j == 0:
                                nc.vector.tensor_copy(
                                    out=st['y'][:, dc * 512:(dc + 1) * 512],
                                    in_=st['ps'])
                                if dc == 1:
                                    nc.sync.dma_start(
                                        out=y[t * 128:(t + 1) * 128, :],
                                        in_=st['y'])
                        yield item

            queue = []

            def pop(n):
                for _ in range(min(n, len(queue))):
                    queue.pop(0)()

            # ---- prologue: kT[0] full + qT[0] first half, inline ----
            for qc in range(4):
                for it in qk_chunk(wk_sb, kT[0], 0, qc, {}, "kT0"):
                    it()
            for qc in range(2):
                for it in qk_chunk(wq_sb, qT[0], 0, qc, {}, "qT0"):
                    it()

            # queue: V-proj, QK pairs 1-3 (kT full + qT first half),
            # then deferred qT second halves (needed only for qh=1)
            for t in range(NTT):
                queue.extend(vproj_tile(t, {}))
            for p in range(1, PAIRS):
                for qc in range(4):
                    queue.extend(qk_chunk(wk_sb, kT[p], p, qc, {}, f"kT{p}"))
                for qc in range(2):
                    queue.extend(qk_chunk(wq_sb, qT[p], p, qc, {}, f"qT{p}"))
            for p in (3, 2, 1, 0):
                for qc in range(2, 4):
                    queue.extend(qk_chunk(wq_sb, qT[p], p, qc, {}, f"qT{p}b"))

            # ---- attention units ----
            for qh in range(2):
                if qh == 1:
                    for t in range(NTT // 2):
                        queue.extend(outproj_tile(t, {}))
                q0 = qh * HALF
                pops = pops_qh0 if qh == 0 else pops_qh1
                for u in range(2 * PAIRS):
                    uu = u if qh == 0 else 2 * PAIRS - 1 - u
                    p, hl = uu // 2, uu % 2
                    po = hl * 64
                    o_ps = ps_o.tile([D_HEAD + 1, HALF], F32, tag="o",
                                     name=f"o_{p}_{hl}_{qh}")
                    e_prev = None
                    for kt_i in range(KT):
                        s_ps = ps_s.tile([128, HALF], F32, tag="s",
                                         name=f"s_{u}_{qh}_{kt_i}")
                        for c in range(2):
                            nc.tensor.matmul(
                                out=s_ps[:, c * 512:(c + 1) * 512],
                                lhsT=kT[p][po:po + 64,
                                           kt_i * 128:(kt_i + 1) * 128],
                                rhs=qT[p][po:po + 64,
                                          q0 + c * 512:q0 + (c + 1) * 512],
                                start=True, stop=True)
                        et = ep.tile([128, HALF], BF16, tag="e",
                                     name=f"e_{u}_{qh}_{kt_i}")
                        nc.scalar.activation(
                            out=et, in_=s_ps,
                            func=mybir.ActivationFunctionType.Exp,
                            scale=SCALE)
                        pop(pops[u])
                        if e_prev is not None:
                            for c in range(2):
                                nc.tensor.matmul(
                                    out=o_ps[:, c * 512:(c + 1) * 512],
                                    lhsT=vb[kt_i - 1][:, 2 * p + hl, :],
                                    rhs=e_prev[:, c * 512:(c + 1) * 512],
                                    start=(kt_i == 1), stop=False)
                        e_prev = et
                    for c in range(2):
                        nc.tensor.matmul(
                            out=o_ps[:, c * 512:(c + 1) * 512],
                            lhsT=vb[KT - 1][:, 2 * p + hl, :],
                            rhs=e_prev[:, c * 512:(c + 1) * 512],
                            start=False, stop=True)
                    # copy O' to SBUF fast to free o_ps, then normalize
                    ot = npool.tile([65, HALF], F32, tag="ot",
                                    name=f"ot_{p}_{hl}_{qh}")
                    nc.vector.tensor_copy(out=ot, in_=o_ps)
                    r1 = npool.tile([1, HALF], F32, tag="r1",
                                    name=f"r1_{p}_{hl}_{qh}")
                    nc.sync.dma_start(out=r1, in_=ot[64:65, :])
                    r2 = npool.tile([1, HALF], F32, tag="r2",
                                    name=f"r2_{p}_{hl}_{qh}")
                    nc.vector.reciprocal(out=r2, in_=r1)
                    rb = npool.tile([64, HALF], F32, tag="rb",
                                    name=f"rb_{p}_{hl}_{qh}")
                    nc.gpsimd.partition_broadcast(rb, r2)
                    if hl == 0:
                        nc.vector.tensor_mul(
                            aT[p][0:64, q0:q0 + HALF], ot[0:64, :], rb)
                    else:
                        tmpb = npool.tile([64, HALF], BF16, tag="tmpb",
                                          name=f"tmpb_{p}_{qh}")
                        nc.vector.tensor_mul(tmpb, ot[0:64, :], rb)
                        nc.sync.dma_start(
                            out=aT[p][64:128, q0:q0 + HALF], in_=tmpb)

            # ---- epilogue: out-projection for qh=1 tokens ----
            for t in range(NTT // 2, NTT):
                queue.extend(outproj_tile(t, {}))
            pop(len(queue))

    nc.compile()
    return nc


def kernel(x, w_qkv, w_out, b_out):
    x = np.asarray(x, dtype=np.float32)
    w_qkv = np.asarray(w_qkv, dtype=np.float32)
    w_out = np.asarray(w_out, dtype=np.float32)
    b_out = np.asarray(b_out, dtype=np.float32)

    if N_TOK not in _NC_CACHE:
        _NC_CACHE[N_TOK] = build_kernel(N_TOK)
    nc = _NC_CACHE[N_TOK]

    core_ids = list(range(8))
    in_maps = _make_in_maps(x, w_qkv, w_out)
    res = run_bass_kernel_spmd(nc, in_maps, core_ids)
    out = np.empty((B, N_TOK, DIM), dtype=np.float32)
    for b in range(B):
        out[b] = res.results[2 * b]["y"] + res.results[2 * b + 1]["y"] + b_out
    return out


def _make_in_maps(x, w_qkv, w_out):
    in_maps = []
    for c in range(8):
        b, g = c // 2, c % 2
        sl = slice(g * INNER_G, (g + 1) * INNER_G)
        in_maps.append({
            "xt": np.ascontiguousarray(x[b].T).astype(BF16_NP),
            "wq": np.ascontiguousarray(
                w_qkv[:, 0 * DIM + sl.start:0 * DIM + sl.stop]).astype(BF16_NP),
            "wk": np.ascontiguousarray(
                w_qkv[:, 1 * DIM + sl.start:1 * DIM + sl.stop]).astype(BF16_NP),
            "wv": np.ascontiguousarray(
                w_qkv[:, 2 * DIM + sl.start:2 * DIM + sl.stop]).astype(BF16_NP),
            "wo": np.ascontiguousarray(w_out[sl]).astype(BF16_NP),
        })
    return in_maps
